# revision 29
# baseline (speedup 1.0000x reference)
"""AxisAttention kernel for 8 TRN2 NeuronCores.

Math (per batch b, per slice c in [0,64) with c = head*16 + hd_channel):
  qkv = W_qkv @ x_b (per pixel 1x1 conv) + b_qkv
  att_c[i,j] = sum_h q_c[h,i] k_c[h,j]              (contract over h)
  att = softmax((att + pos_c) * gamma, axis=j)
  net_c[h,j] = sum_i v_c[h,i] att[i,j]              (contract over i=w)

Design:
  - Data parallel: 1 batch per core, 8 cores, no collectives.
  - Two passes through DRAM spill buffers: pass 1 projects (channels on
    partitions, pixels on free dim) and writes qkv slice-major; pass 2
    reads per-slice (h, w) maps (the DRAM round trip performs the
    (ch,pix)->(h,w) partition transposition for free).
  - Spill is bf16 (halves spill traffic; rel err ~3e-3).
  - pos is never materialized: pos_c[i,j] = sin/cos(a_i + b_j) is rank-2
    by angle addition, folded into the attention matmul as 2 extra
    contraction rows with host-precomputed factors (loaded once, bf16).
  - Softmax needs no max subtraction (logits are bounded ~ +-2) and the
    ACT Exp writes psum->sbuf with scale=gamma, its accum_out giving the
    row sum for free. The 1/rowsum is folded into the vT psum->sbuf
    evacuation (scales v's contraction columns), so the exp output is
    used unnormalized.
  - v psum tiles are paired into [128, 512] so the ACT evacuation count
    halves; q+k spill is one [128, CHUNK] DMA per chunk.
  - DMA issue queues: reads on SP (sync), spill/out writes on Pool
    (SWDGE) so waiting DMAs never block ACT/DVE compute queues.
"""

import sys

sys.path.insert(0, "/opt/trn_rl_repo")

import numpy as np

DIM = 64
NUM_HEAD = 4
SEQ = 256
HW = SEQ * SEQ  # 65536
BASE = 10000.0
GAMMA = (DIM // NUM_HEAD) ** (-0.5)  # 0.25
N_CORES = 8

SG = 8  # pass-2 slice group size (slices per DMA batch)
import os as _os

PASSES = int(_os.environ.get("KERNEL_PASSES", "3"))  # bit0: pass1, bit1: pass2
QK_EVAC_ACT = 4  # of the 16 qk evacs per chunk, how many go to ACT

_CACHE = {}


def _pos_factors():
    """Rank-2 factorization of the additive positional map.

    pos[c, i, j] = emb[i*SEQ + j, c] with, for t = c // 2:
      c even: sin((i*SEQ + j) * inv_t)
      c odd:  cos((i*SEQ + j) * inv_t)
    With a = i*SEQ*inv_t, b = j*inv_t:
      sin(a+b) = sin a * cos b + cos a * sin b
      cos(a+b) = cos a * cos b + (-sin a) * sin b
    Returns posu, posw (64, 2, 256) float32 with pos[c] = posu[c].T @ posw[c].
    """
    half = DIM // 2
    inv = np.power(BASE, -2.0 * np.arange(half, dtype=np.float64) / DIM)
    i_idx = np.arange(SEQ, dtype=np.float64)
    posu = np.zeros((DIM, 2, SEQ), dtype=np.float64)
    posw = np.zeros((DIM, 2, SEQ), dtype=np.float64)
    two_pi = 2.0 * np.pi
    for c in range(DIM):
        t = c // 2
        a = np.mod(i_idx * SEQ * inv[t], two_pi)
        b = np.mod(i_idx * inv[t], two_pi)
        sa, ca = np.sin(a), np.cos(a)
        sb, cb = np.sin(b), np.cos(b)
        if c % 2 == 0:
            posu[c, 0], posu[c, 1] = sa, ca
        else:
            posu[c, 0], posu[c, 1] = ca, -sa
        posw[c, 0], posw[c, 1] = cb, sb
    return posu.astype(np.float32), posw.astype(np.float32)


def _build_nc(reps=1):
    import concourse.tile as tile
    from concourse import bacc, mybir
    from concourse.mybir import ActivationFunctionType as AF

    f32 = mybir.dt.float32
    bf16 = mybir.dt.bfloat16
    sdt = bf16  # spill / attention dtype
    xdt = bf16
    odt = bf16

    nc = bacc.Bacc(None, target_bir_lowering=False, debug=False)

    # x2[s*64 + c, ch*(CHUNK//2) + j] = x[c, ch*CHUNK + s*(CHUNK//2) + j]
    # (two half-chunks stacked on partitions so x DMAs span 128 partitions)
    x_d = nc.declare_dram_parameter("x", [128, HW // 2], xdt, isOutput=False)
    wqk_d = nc.declare_dram_parameter("wqk", [128, 128], xdt, isOutput=False)
    wv_d = nc.declare_dram_parameter("wv", [128, 64], xdt, isOutput=False)
    bqk_d = nc.declare_dram_parameter("bqk", [128, 1], f32, isOutput=False)
    bv_d = nc.declare_dram_parameter("bv", [128, 1], f32, isOutput=False)
    posuw_d = nc.declare_dram_parameter("posuw", [2, 2, DIM, SEQ], bf16, isOutput=False)
    ident_d = nc.declare_dram_parameter("ident", [128, 128], sdt, isOutput=False)
    out_d = nc.declare_dram_parameter("out", [DIM, HW], odt, isOutput=True)

    CHUNK = 16384  # pixels per pass-1 outer chunk
    SUB = 512  # pixels per projection matmul
    NSUB = CHUNK // SUB
    NCHUNK = HW // CHUNK

    with tile.TileContext(nc) as tc:
        with (
            tc.tile_pool(name="consts", bufs=1) as consts,
            tc.tile_pool(name="dram", bufs=1, space="DRAM") as dramp,
        ):
            spill = dramp.tile([3, DIM, HW], sdt)  # [q/k/v, c, pix]

            wqk = consts.tile([128, 128], xdt)
            wv = consts.tile([128, 64], xdt)
            bqk = consts.tile([128, 1], f32)
            bv = consts.tile([128, 1], f32)
            ident = consts.tile([128, 128], sdt)
            nc.sync.dma_start(out=wqk[:], in_=wqk_d[:])
            nc.sync.dma_start(out=wv[:], in_=wv_d[:])
            nc.sync.dma_start(out=bqk[:], in_=bqk_d[:])
            nc.sync.dma_start(out=bv[:], in_=bv_d[:])
            nc.sync.dma_start(out=ident[:], in_=ident_d[:])

            for _rep in range(reps):
                # ------------ pass 1: QKV projection -> DRAM spill ----------
                if PASSES & 1:
                    with (
                        tc.tile_pool(name="p1x", bufs=2) as p1x,
                        tc.tile_pool(name="p1s", bufs=2) as p1s,
                        tc.tile_pool(name="p1ps", bufs=3, space="PSUM") as p1ps,
                    ):
                        HC = CHUNK // 2
                        xts = [None] * NCHUNK
                        xts[0] = p1x.tile([128, HC], xdt, tag="xt", name="xt0")
                        nc.sync.dma_start(out=xts[0][:], in_=x_d[:, 0:HC])
                        for ch in range(NCHUNK):
                            pix0 = ch * CHUNK
                            xt = xts[ch]
                            if ch + 1 < NCHUNK:
                                # prefetch next chunk ahead of this chunk's
                                # v spill write on the SP queue
                                xts[ch + 1] = p1x.tile(
                                    [128, HC], xdt, tag="xt", name=f"xt{ch + 1}"
                                )
                                nc.sync.dma_start(
                                    out=xts[ch + 1][:],
                                    in_=x_d[:, (ch + 1) * HC : (ch + 2) * HC],
                                )
                            qk_st = p1s.tile([128, CHUNK], sdt)
                            v_st = p1s.tile([128, CHUNK // 2], sdt)
                            # (sp, half): pixel block half*HC + sp*SUB; the
                            # v psum pairs rows 0:64 (half 0) with 64:128
                            # (half 1) so its evacuation is [128, SUB] and
                            # the v spill is one 128-partition DMA.
                            for sp in range(NSUB // 2):
                                for half in range(2):
                                    b0 = half * 64
                                    col = sp * SUB
                                    xs = xt[b0 : b0 + 64, col : col + SUB]
                                    ps_qk = p1ps.tile([128, SUB], f32, tag="psqk")
                                    if half == 0:
                                        ps_v = p1ps.tile([128, SUB], f32, tag="psv")
                                    nc.tensor.matmul(
                                        ps_qk[:], wqk[b0 : b0 + 64, :], xs
                                    )
                                    nc.tensor.matmul(
                                        ps_v[b0 : b0 + 64, :],
                                        wv[b0 : b0 + 64, :],
                                        xs,
                                    )
                                    qcol = half * (CHUNK // 2) + sp * SUB
                                    # psum -> sbuf (+bias, +cast): qk on DVE
                                    # with every 4th on ACT
                                    if (sp * 2 + half) % 4 == 3:
                                        nc.scalar.activation(
                                            out=qk_st[:, qcol : qcol + SUB],
                                            in_=ps_qk[:],
                                            func=AF.Identity,
                                            bias=bqk[:],
                                        )
                                    else:
                                        nc.vector.tensor_scalar_add(
                                            qk_st[:, qcol : qcol + SUB],
                                            ps_qk[:],
                                            bqk[:],
                                        )
                                    if half == 1:
                                        nc.scalar.activation(
                                            out=v_st[:, col : col + SUB],
                                            in_=ps_v[:],
                                            func=AF.Identity,
                                            bias=bv[:],
                                        )
                            # q+k spill: one 128-partition DMA (qk_st rows
                            # 0:64 = q, 64:128 = k; (s c) merges since the
                            # spill is s-major)
                            nc.gpsimd.dma_start(
                                out=spill[0:2, :, pix0 : pix0 + CHUNK].rearrange(
                                    "s c w -> (s c) w"
                                ),
                                in_=qk_st[:],
                            )
                            # v spill: two half-chunk DMAs
                            nc.sync.dma_start(
                                out=spill[2, :, pix0 : pix0 + HC],
                                in_=v_st[0:64, :],
                            )
                            nc.sync.dma_start(
                                out=spill[2, :, pix0 + HC : pix0 + CHUNK],
                                in_=v_st[64:128, :],
                            )

                # ------------ pass 2: per-slice attention -------------------
                if PASSES & 2:
                    with (
                        tc.tile_pool(name="p2io", bufs=3) as p2io,
                        tc.tile_pool(name="p2w", bufs=3) as p2w,
                        tc.tile_pool(name="p2ps", bufs=2, space="PSUM") as p2ps,
                    ):
                        for c0 in range(0, DIM, SG):
                            pg = p2w.tile([2, 2, SG, SEQ], bf16, tag="pg")
                            nc.gpsimd.dma_start(
                                out=pg[:], in_=posuw_d[:, :, c0 : c0 + SG, :]
                            )
                            # qkvg[p, s, cc*2 + g, w], h = g*128 + p;
                            # one 128-partition DMA per q/k/v plane
                            qkvg = p2io.tile([128, 3, 2 * SG, SEQ], sdt, tag="qkvg")
                            for s3 in range(3):
                                eng3 = nc.gpsimd if s3 == 1 else nc.sync
                                eng3.dma_start(
                                    out=qkvg[:, s3, :, :],
                                    in_=spill[s3].rearrange(
                                        "c (g p w) -> p (c g) w", g=2, p=128
                                    )[:, 2 * c0 : 2 * (c0 + SG), :],
                                )
                            og = p2io.tile([128, 2 * SG, SEQ], odt, tag="og")

                            for cc in range(SG):
                                c = c0 + cc
                                e = p2w.tile([128, 2, SEQ], sdt, tag="e")
                                rs = p2w.tile([128, 2], f32, tag="rs")
                                rr = p2w.tile([128, 2], f32, tag="rr")
                                for ic in range(2):
                                    ps_att = p2ps.tile([128, SEQ], f32, tag="psatt")
                                    isl = slice(ic * 128, (ic + 1) * 128)
                                    nc.tensor.matmul(
                                        ps_att[:],
                                        qkvg[:, 0, cc * 2 + 0, isl],
                                        qkvg[:, 1, cc * 2 + 0, :],
                                        start=True,
                                        stop=False,
                                    )
                                    nc.tensor.matmul(
                                        ps_att[:],
                                        qkvg[:, 0, cc * 2 + 1, isl],
                                        qkvg[:, 1, cc * 2 + 1, :],
                                        start=False,
                                        stop=False,
                                    )
                                    nc.tensor.matmul(
                                        ps_att[:],
                                        pg[:, 0, cc, isl],
                                        pg[:, 1, cc, :],
                                        start=False,
                                        stop=True,
                                    )
                                    nc.scalar.activation(
                                        out=e[:, ic, :],
                                        in_=ps_att[:],
                                        func=AF.Exp,
                                        scale=GAMMA,
                                        accum_out=rs[:, ic : ic + 1],
                                    )
                                nc.vector.reciprocal(rr[:], rs[:])

                                # vT[p, wc, h] = v[c, h, wc*128+p] / r[wc*128+p]
                                # (1/rowsum folded into the psum evacuation;
                                # both g-halves transposed into one psum tile
                                # so the evacuation is a single [128,256] op)
                                vT = p2w.tile([128, 2, SEQ], sdt, tag="vT")
                                for wc in range(2):
                                    ps_t = p2ps.tile([128, SEQ], sdt, tag="pst")
                                    for g in range(2):
                                        nc.tensor.transpose(
                                            ps_t[:, g * 128 : (g + 1) * 128],
                                            qkvg[
                                                :,
                                                2,
                                                cc * 2 + g,
                                                wc * 128 : (wc + 1) * 128,
                                            ],
                                            ident[:],
                                        )
                                    nc.vector.tensor_scalar_mul(
                                        vT[:, wc, :],
                                        ps_t[:],
                                        rr[:, wc : wc + 1],
                                    )

                                # net[h,j] = sum_i v[h,i]/r[i] * e[i,j]
                                for m in range(2):
                                    ps_net = p2ps.tile([128, SEQ], f32, tag="psnet")
                                    msl = slice(m * 128, (m + 1) * 128)
                                    nc.tensor.matmul(
                                        ps_net[:],
                                        vT[:, 0, msl],
                                        e[:, 0, :],
                                        start=True,
                                        stop=False,
                                    )
                                    nc.tensor.matmul(
                                        ps_net[:],
                                        vT[:, 1, msl],
                                        e[:, 1, :],
                                        start=False,
                                        stop=True,
                                    )
                                    if (cc * 2 + m) % 4 == 3:
                                        nc.scalar.activation(
                                            out=og[:, cc * 2 + m, :],
                                            in_=ps_net[:],
                                            func=AF.Identity,
                                        )
                                    else:
                                        nc.vector.tensor_copy(
                                            og[:, cc * 2 + m, :], ps_net[:]
                                        )
                            nc.gpsimd.dma_start(
                                out=out_d[:]
                                .rearrange("c (m p w) -> p (c m) w", m=2, p=128)[
                                    :, 2 * c0 : 2 * (c0 + SG), :
                                ],
                                in_=og[:],
                            )

    nc.compile()
    return nc


def _get_nc(reps=1):
    key = ("nc", reps)
    if key not in _CACHE:
        _CACHE[key] = _build_nc(reps)
    return _CACHE[key]


def _make_in_maps(x, W_qkv, b_qkv):
    import ml_dtypes

    posu, posw = _pos_factors()
    posuw = np.stack(
        [posu.transpose(1, 0, 2), posw.transpose(1, 0, 2)], axis=1
    )  # (2, 2, 64, 256)
    posuw = np.ascontiguousarray(posuw).astype(ml_dtypes.bfloat16)
    wdt = ml_dtypes.bfloat16
    # weights replicated on partitions 64:128 (matmul needs lhsT and rhs at
    # the same base partition; x tiles stack two half-chunks on partitions)
    wqk1 = W_qkv[0:128].T.astype(wdt)  # (64, 128)
    wv1 = W_qkv[128:192].T.astype(wdt)  # (64, 64)
    wqk = np.ascontiguousarray(np.vstack([wqk1, wqk1]))  # (128, 128)
    wv = np.ascontiguousarray(np.vstack([wv1, wv1]))  # (128, 64)

    bqk = np.ascontiguousarray(b_qkv[0:128].astype(np.float32)).reshape(128, 1)
    bv1 = b_qkv[128:192].astype(np.float32)
    bv = np.ascontiguousarray(np.concatenate([bv1, bv1])).reshape(128, 1)
    ident = np.eye(128, dtype=np.float32).astype(ml_dtypes.bfloat16)
    NCHUNK, CHUNK = 4, 16384
    in_maps = []
    for b in range(N_CORES):
        xb = np.asarray(x[b]).reshape(DIM, NCHUNK, 2, CHUNK // 2)
        x2 = np.ascontiguousarray(xb.transpose(2, 0, 1, 3)).reshape(128, HW // 2)
        in_maps.append(
            {
                "x": x2.astype(ml_dtypes.bfloat16),
                "wqk": wqk,
                "wv": wv,
                "bqk": bqk,
                "bv": bv,
                "posuw": posuw,
                "ident": ident,
            }
        )
    return in_maps


def kernel(x, W_qkv, b_qkv, pos):
    from concourse.bass_utils import run_bass_kernel_spmd

    x = np.asarray(x, dtype=np.float32)
    W_qkv = np.asarray(W_qkv, dtype=np.float32)
    b_qkv = np.asarray(b_qkv, dtype=np.float32)
    assert x.shape == (N_CORES, DIM, SEQ, SEQ)

    nc = _get_nc()
    in_maps = _make_in_maps(x, W_qkv, b_qkv)
    res = run_bass_kernel_spmd(nc, in_maps, core_ids=list(range(N_CORES)))
    out = np.stack(
        [
            np.asarray(res.results[b]["out"], dtype=np.float32).reshape(
                DIM, SEQ, SEQ
            )
            for b in range(N_CORES)
        ]
    )
    return out.astype(np.float32)


if __name__ == "__main__":
    nc = _get_nc()
    print("built ok")
